# revision 60
# baseline (speedup 1.0000x reference)
"""Multi-head causal self-attention (B=4, T=2048, C=1024, 16 heads) on 8 trn2 cores.

Sharding: data-parallel over batch (4) x tensor-parallel over heads (2 groups of 8).
Core m handles batch m//2, head group m%2. Host pre-transposes x and the weights
(bf16) so every on-device matmul consumes operands in natural layout; the output
projection partial sums are pair-reduced on host (+bias).

v2 pipeline (bf16 matmuls, fp32 PSUM):
  - QKV projection strips interleaved with attention query-blocks at single-MM
    granularity: projection matmuls fill the PE bubbles that the ACT-bound
    softmax stretches would otherwise leave.
  - scores for a head pair land in the two banks of one [128,1024] PSUM tile;
    ONE activation (2D AP) exponentiates both banks -> halves ACT inst count.
  - causal masking: exp the unmasked column range, then gpsimd affine_select
    zeroes the diagonal triangle in the bf16 p tile (Pool engine is idle).
  - softmax denominators via a ones-column in v; normalization r=1/denom via
    DVE reciprocal, broadcast across partitions with a free-dim-stride-0 DMA,
    then one fused PSUM*SBUF->bf16 multiply per head writes avT in place
    (par1 writes partitions 64:128 directly - no staging DMA). The very last
    block swaps the DMA broadcast for a ones-matmul into the freed scores
    ring (shorter latency on the kernel's critical tail).
  - output projection chains rotate their accumulation order (c4 1,2,3,0 on
    the last query block) so only one matmul per chain gates on the final
    normalize; outputs stage through SBUF (DMA cannot read PSUM) with copies
    alternating ACT/DVE at the tail.
"""

import numpy as np

import concourse.bass as bass
import concourse.mybir as mybir
import concourse.tile as tile
from concourse.bass_utils import run_bass_kernel_spmd

F32 = mybir.dt.float32
F32R = mybir.dt.float32r
BF16 = mybir.dt.bfloat16
AF = mybir.ActivationFunctionType
MULT = mybir.AluOpType.mult

B, T, C = 4, 2048, 1024
HEADS, D = 16, 64
GROUPS = 2                  # head groups (tensor parallel)
HPC = HEADS // GROUPS       # heads per core = 8
GC = HPC * D                # group channel width = 512
CCH = C // 128              # contraction chunks = 8
NSTRIP = T // 512           # strips / query blocks = 4
HP2 = HPC // 2              # head pairs = 4

_PROGRAM = None


def _patch_drain_chunking():
    """The axon walrus build rejects instructions with >~4 sem waits; Tile's
    kernel-tail drain waits on every live semaphore at once. Split it into a
    chain of drains (excess waits then move onto NoOps via
    _split_excess_waits, keeping every instruction at <=1 wait)."""
    from bass_rust import VectorClock, ScopedClock

    if getattr(tile.TileContext, "_drain_chunk_patched", False):
        return

    def _drain_and_barrier(self, tick_clock, wait_clock):
        gc_vec = list(tick_clock.global_clock)
        nz = [i for i, t in enumerate(gc_vec) if t > 0]
        CHUNK = 32
        for k in range(0, len(nz), CHUNK):
            keep = set(nz[k:k + CHUNK])
            partial = [gc_vec[i] if i in keep else 0 for i in range(len(gc_vec))]
            d = self.nc.sync.drain()
            wait_clock.add_sem_waits(d.ins, ScopedClock({None: VectorClock(partial)}))
        self.nc.all_engine_barrier()
        assert self.sems is not None
        popped = self.nc._tile_sem_poison_stack.pop()
        assert popped is self._sem_poison
        self.nc.clear_and_free_semaphores(list(self.sems.allocated().values()))
        self.nc.all_engine_barrier()

    tile.TileContext._drain_and_barrier = _drain_and_barrier
    tile.TileContext._drain_chunk_patched = True


def _split_excess_waits(nc, maxw=1, maxw_other=None):
    """Walrus rejects instructions carrying more than ~1 sem wait. Move excess
    waits onto same-engine NoOps inserted immediately before the instruction
    (engine streams execute in bb order, so semantics are preserved)."""
    from bass_rust import InstNoOp

    ctr = 0
    for f in nc.m.functions:
        for bb in f.blocks:
            new_insts = []
            for inst in bb.instructions:
                si = inst.sync_info
                waits = list(si.on_wait) if si and si.on_wait else []
                lim = maxw
                if maxw_other is not None and str(inst.engine) != 'EngineType.PE':
                    lim = maxw_other
                maxw_eff = lim
                if len(waits) > maxw_eff:
                    head, rest = waits[:-maxw_eff], waits[-maxw_eff:]
                    for k in range(0, len(head), maxw_eff):
                        ctr += 1
                        new_insts.append(InstNoOp(
                            name=f"waitnop_{ctr}",
                            engine=inst.engine,
                            sync_info=mybir.SyncInfo(
                                on_wait=head[k:k + maxw_eff], on_update=[]),
                        ))
                    inst.sync_info = mybir.SyncInfo(on_wait=rest, on_update=si.on_update)
                new_insts.append(inst)
            bb.instructions = new_insts
    return ctr


def _build_program(split_waits=True):
    _patch_drain_chunking()
    nc = bass.Bass()

    xT_d = nc.declare_dram_parameter("xT", [C, T], BF16, isOutput=False)
    wq_d = nc.declare_dram_parameter("wqT", [C, GC], BF16, isOutput=False)
    wk_d = nc.declare_dram_parameter("wkT", [C, GC], BF16, isOutput=False)
    wv_d = nc.declare_dram_parameter("wvT", [C, GC], BF16, isOutput=False)
    wp_d = nc.declare_dram_parameter("wpT", [GC, C], BF16, isOutput=False)
    out_d = nc.declare_dram_parameter("outp", [T, C], F32, isOutput=True)

    from collections import deque
    from contextlib import ExitStack

    with tile.TileContext(nc) as tc, ExitStack() as stack:
        pers = stack.enter_context(tc.tile_pool(name="pers", bufs=1))
        qT = pers.tile([128, HP2, T], BF16, tag="qT")     # [chan-in-pair, hp, t]
        kT = pers.tile([128, HP2, T], BF16, tag="kT")
        # v with a trailing ones column per head: [key-chunk, head, D+1]
        v = pers.tile([128, T // 128, HPC, D + 1], BF16, tag="v")
        avT = pers.tile([128, HP2, T], BF16, tag="avT")
        wq = pers.tile([128, CCH, GC], BF16, tag="wq")
        wk = pers.tile([128, CCH, GC], BF16, tag="wk")
        wv = pers.tile([128, CCH, GC], BF16, tag="wv")
        wp = pers.tile([128, GC // 128, C], BF16, tag="wp")

        nc.gpsimd.memset(v[:, :, :, D:D + 1], 1.0)
        # walrus: every writer of a tensor consumed by an f32r matmul must
        # itself round to f32r (ACT/DMA only) -> keep Pool/DVE-written scratch
        # in separate tiles from the f32r-consumed ones
        ones_src = pers.tile([128, 64], F32, tag="ones_src")
        nc.gpsimd.memset(ones_src[64:65, :], 1.0)
        ones = pers.tile([128, 64], F32, tag="ones")
        nc.scalar.activation(ones[64:65, :].bitcast(F32R), ones_src[64:65, :],
                             AF.Copy, scale=1.0)

        warm_w = pers.tile([128, 64], BF16, tag="warm_w")
        nc.gpsimd.memset(warm_w[:, :], 0.0)

        xs_pool = stack.enter_context(tc.tile_pool(name="xs", bufs=4))
        pt_pool = stack.enter_context(tc.tile_pool(name="pt", bufs=6))
        rb_pool = stack.enter_context(tc.tile_pool(name="rb", bufs=3))
        ob_pool = stack.enter_context(tc.tile_pool(name="ob", bufs=4))
        ps = stack.enter_context(tc.tile_pool(name="ps", bufs=1, space="PSUM"))

        # ---- initial DMAs ----
        xs_tiles = [None] * NSTRIP

        def load_strip(s):
            xs = xs_pool.tile([128, CCH, 512], BF16, tag="xs", name=f"xs{s}")
            nc.sync.dma_start(
                xs[:, :, :],
                xT_d[:, 512 * s:512 * (s + 1)].rearrange("(c p) t -> p c t", p=128))
            xs_tiles[s] = xs

        # first-use order, with wk/xs0 halved so the k chain starts ASAP
        xs0 = xs_pool.tile([128, CCH, 512], BF16, tag="xs", name="xs0")
        xT0 = xT_d[:, 0:512].rearrange("(c p) t -> p c t", p=128)
        wkT0 = wk_d[:, :].rearrange("(c p) o -> p c o", p=128)
        wqT0 = wq_d[:, :].rearrange("(c p) o -> p c o", p=128)
        nc.sync.dma_start(wk[:, 0:4, :], wkT0[:, 0:4, :])
        nc.sync.dma_start(xs0[:, 0:4, :], xT0[:, 0:4, :])
        nc.sync.dma_start(wk[:, 4:8, :], wkT0[:, 4:8, :])
        nc.sync.dma_start(xs0[:, 4:8, :], xT0[:, 4:8, :])
        nc.sync.dma_start(wq[:, 0:4, :], wqT0[:, 0:4, :])
        nc.sync.dma_start(wq[:, 4:8, :], wqT0[:, 4:8, :])
        xs_tiles[0] = xs0
        # keep the PE pstate warm through the initial load: tiny matmuls that
        # each consume a just-landed DMA chunk (same reason real kernels avoid
        # >3us PE-idle gaps: the clock ramps back down)
        warmav = ps.tile([128, 1024], F32, tag="av", bufs=1, name="warmav")
        for wsrc in (wk[:, 0, 0:64], xs0[:, 0, 0:64], wk[:, 4, 0:64],
                     xs0[:, 4, 0:64], wq[:, 0, 0:64], wq[:, 4, 0:64]):
            nc.tensor.matmul(warmav[0:64, 0:64], warm_w[:, :], wsrc,
                             start=True, stop=True)
        wvT0 = wv_d[:, :].rearrange("(c p) o -> p c o", p=128)
        nc.sync.dma_start(wv[:, 0:4, :], wvT0[:, 0:4, :])
        nc.sync.dma_start(wv[:, 4:8, :], wvT0[:, 4:8, :])
        nc.sync.dma_start(wp[:, :, :],
                          wp_d[:, :].rearrange("(c p) o -> p c o", p=128))

        # ---- phase-1 / phase-3 micro-item generators (PE filler) ----
        def qk_chain_items(s, w_sb, dst, o):
            cell = {}
            def mk_mm(c):
                def it():
                    if c == 0:
                        cell["p"] = ps.tile([128, 512], F32, tag="p1", bufs=2, name="pq")
                    nc.tensor.matmul(cell["p"][:, :], w_sb[:, c, 128 * o:128 * (o + 1)],
                                     xs_tiles[s][:, c, :],
                                     start=(c == 0), stop=(c == CCH - 1))
                return it
            for c in range(CCH):
                yield mk_mm(c)
            def cp():
                nc.vector.tensor_copy(dst[:, o, 512 * s:512 * (s + 1)], cell["p"][:, :])
            yield cp

        def v_chain_items(s, tt):
            cell = {}
            def mk_mm(c):
                def it():
                    if c == 0:
                        cell["p"] = ps.tile([128, 512], F32, tag="p1", bufs=2, name="pv")
                    nc.tensor.matmul(cell["p"][:, :], xs_tiles[s][:, c, 128 * tt:128 * (tt + 1)],
                                     wv[:, c, :], start=(c == 0), stop=(c == CCH - 1))
                return it
            for c in range(CCH):
                yield mk_mm(c)
            def cp():
                # ACT copy (same act table as Exp): keeps DVE free during the
                # strip dumps where these chains run back-to-back
                nc.scalar.activation(
                    v[:, 4 * s + tt, :, 0:D],
                    cell["p"][:, :].rearrange("p (h d) -> p h d", h=HPC),
                    AF.Copy, scale=1.0)
            yield cp

        def strip_items(s):
            # hp0's k/q chains + all v chains first (attention j=s, hp=0 needs
            # them); later head pairs' chains trail as PE filler with per-hp
            # barrier markers. Strip 0 front-loads the k chains: they only
            # need wk+xs0, so they fill the PE while wq/wv are still in DMA.
            if s == 0:
                yield from qk_chain_items(s, wk, kT, 0)
                yield from qk_chain_items(s, wk, kT, 1)
                yield from qk_chain_items(s, wq, qT, 0)
                yield from qk_chain_items(s, wq, qT, 1)
                for tt in range(4):
                    yield from v_chain_items(s, tt)
                yield "s0hp0"
                yield "s0hp1"
                for o in range(2, HP2):
                    yield from qk_chain_items(s, wk, kT, o)
                    yield from qk_chain_items(s, wq, qT, o)
                    yield f"s0hp{o}"
                return
            yield from qk_chain_items(s, wk, kT, 0)
            yield from qk_chain_items(s, wq, qT, 0)
            for tt in range(4):
                yield from v_chain_items(s, tt)
            yield f"s{s}hp0"
            for o in range(1, HP2):
                yield from qk_chain_items(s, wk, kT, o)
                yield from qk_chain_items(s, wq, qT, o)
                yield f"s{s}hp{o}"

        def po_items(j, c4_order=(0, 1, 2, 3), act_copies=False, tts=None):
            for tt in (tts if tts is not None else range(4 * j, 4 * (j + 1))):
                for o2 in range(2):
                    cell = {}
                    def mk_mm(ci, c4, tt=tt, o2=o2, cell=cell):
                        def it():
                            if ci == 0:
                                cell["p"] = ps.tile([128, 512], F32, tag="p1",
                                                    bufs=2, name="po")
                            nc.tensor.matmul(cell["p"][:, :],
                                             avT[:, c4, 128 * tt:128 * (tt + 1)],
                                             wp[:, c4, 512 * o2:512 * (o2 + 1)],
                                             start=(ci == 0), stop=(ci == GC // 128 - 1))
                        return it
                    for ci, c4 in enumerate(c4_order):
                        yield mk_mm(ci, c4)
                    def cp(cell=cell, tt=tt, o2=o2):
                        cell["ob"] = ob_pool.tile([128, 512], F32, tag="ob", name="ob")
                        if act_copies and (tt + o2) % 2 == 0:
                            nc.scalar.activation(cell["ob"][:, :], cell["p"][:, :],
                                                 AF.Copy, scale=1.0)
                        else:
                            nc.vector.tensor_copy(cell["ob"][:, :], cell["p"][:, :])
                    yield cp
                    def dma(tt=tt, o2=o2, cell=cell):
                        nc.sync.dma_start(
                            out_d[128 * tt:128 * (tt + 1), 512 * o2:512 * (o2 + 1)],
                            cell["ob"][:, :])
                    yield dma

        # PE filler queue. Items run strictly in queue order (chains allocate
        # PSUM ring slots, so partial-chain reordering would deadlock);
        # markers let the schedule force "everything up to X done" barriers.
        filler = deque()
        seen_marks = set()

        def mark(name):
            filler.append(name)

        def pull(n):
            done = 0
            while filler and done < n:
                it = filler.popleft()
                if isinstance(it, str):
                    seen_marks.add(it)
                    continue
                it()
                done += 1

        def flush_until(name):
            while name not in seen_marks:
                assert filler, f"marker {name} never queued"
                it = filler.popleft()
                if isinstance(it, str):
                    seen_marks.add(it)
                else:
                    it()

        def flush():
            while filler:
                it = filler.popleft()
                if isinstance(it, str):
                    seen_marks.add(it)
                else:
                    it()

        # ---- attention for one (j, hp) block ----
        def attention_block(j, hp, fast_norm=False):
            nkc = 4 * (j + 1)
            av = ps.tile([128, 1024], F32, tag="av", bufs=1, name="av")

            def emit_scores_exp(i):
                roff = max(0, 128 * i - 512 * j)
                sp = ps.tile([128, 1024], F32, tag="s", bufs=2, name="sp")
                for par in range(2):
                    nc.tensor.matmul(
                        sp[:, 512 * par + roff:512 * (par + 1)],
                        kT[64 * par:64 * (par + 1), hp, 128 * i:128 * (i + 1)],
                        qT[64 * par:64 * (par + 1), hp, 512 * j + roff:512 * (j + 1)],
                        start=True, stop=True)
                ptile = pt_pool.tile([128, 1024], BF16, tag="pt", name="ptile")
                if roff == 0:
                    nc.scalar.activation(ptile[:, :], sp[:, :], AF.Exp, scale=0.125)
                else:
                    nc.scalar.activation(
                        ptile[:, :].rearrange("p (g c) -> p g c", g=2)[:, :, roff:512],
                        sp[:, :].rearrange("p (g c) -> p g c", g=2)[:, :, roff:512],
                        AF.Exp, scale=0.125)
                if 128 * i >= 512 * j:  # diagonal chunk: zero the triangle above diag
                    sel = ptile[:, :].rearrange("p (g c) -> p g c", g=2)[:, :, roff:roff + 128]
                    nc.gpsimd.affine_select(
                        out=sel, in_=sel,
                        compare_op=mybir.AluOpType.is_ge, fill=0.0, base=0,
                        pattern=[[0, 2], [1, 128]], channel_multiplier=-1)
                return ptile, roff

            def emit_av(i, ptile, roff):
                for par in range(2):
                    nc.tensor.matmul(
                        av[0:D + 1, 512 * par + roff:512 * (par + 1)],
                        v[:, i, 2 * hp + par, :],
                        ptile[:, 512 * par + roff:512 * (par + 1)],
                        start=(i == 0), stop=(i == nkc - 1))

            prev = emit_scores_exp(0)
            for i in range(1, nkc):
                cur = emit_scores_exp(i)
                pull(1)
                emit_av(i - 1, *prev)
                prev = cur
            pull(1)
            emit_av(nkc - 1, *prev)

            # normalize: r = 1/denom (row 64), partition-broadcast, one fused
            # multiply per par writes avT directly
            rbt = rb_pool.tile([128, 1024], F32, tag="rb", name="rbt")
            if fast_norm:
                # tail path: the broadcast DMA's ~3us latency would sit bare
                # on the critical path; use a ones-matmul into the (now idle)
                # scores ring + ACT copy instead, pipelined across pars
                bc = ps.tile([128, 1024], F32, tag="s", bufs=2, name="bc")
                rr = rb_pool.tile([128, 1024], F32, tag="rr", bufs=1, name="rr")
                rf = rb_pool.tile([128, 1024], F32, tag="rf", bufs=1, name="rf")
                cols = [slice(512 * par, 512 * (par + 1)) for par in range(2)]
                for par in range(2):
                    nc.vector.reciprocal(rr[64:65, cols[par]], av[64:65, cols[par]])
                for par in range(2):
                    nc.scalar.activation(rf[64:65, cols[par]].bitcast(F32R),
                                         rr[64:65, cols[par]], AF.Copy, scale=1.0)
                for par in range(2):
                    nc.tensor.matmul(bc[0:64, cols[par]], ones[64:65, :].bitcast(F32R),
                                     rf[64:65, cols[par]].bitcast(F32R),
                                     start=True, stop=True)
                for par in range(2):
                    nc.scalar.activation(rbt[0:64, cols[par]], bc[0:64, cols[par]],
                                         AF.Copy, scale=1.0)
                for par in range(2):
                    nc.vector.tensor_tensor(
                        avT[64 * par:64 * (par + 1), hp, 512 * j:512 * (j + 1)],
                        av[0:D, cols[par]], rbt[0:64, cols[par]], op=MULT)
            else:
                nc.vector.reciprocal(rbt[64:65, :], av[64:65, :])
                nc.sync.dma_start(
                    rbt[0:64, :],
                    rbt[64:65, :].unsqueeze(1).to_broadcast((1, 64, 1024)))
                for par in range(2):
                    nc.vector.tensor_tensor(
                        avT[64 * par:64 * (par + 1), hp, 512 * j:512 * (j + 1)],
                        av[0:D, 512 * par:512 * (par + 1)],
                        rbt[0:64, 512 * par:512 * (par + 1)], op=MULT)

        # ---- schedule ----
        # A(j, hp) flushes the queue up to strip j's hp marker; everything
        # behind that marker (later head pairs' projection chains, the next
        # strip, deferred output-projection chains) serves as PE filler
        # inside the ACT-bound attention stretches.
        filler.extend(strip_items(0))
        load_strip(1)
        filler.extend(strip_items(1))

        # po(0) feeds the A(2)/A(3) pulls; po(1)/po(2) are held back so the
        # PE still has independent work while the last block's normalize
        # chain (reciprocal -> broadcast DMA -> avT multiplies) drains
        for j in range(NSTRIP):
            # at j=3, hp0 goes last: po(3) chains accumulate c4 in order
            # 1,2,3,0 so only their final matmul waits on the last
            # normalize chain of the kernel
            hps = (1, 2, 3, 0) if j == NSTRIP - 1 else range(HP2)
            for i_hp, hp in enumerate(hps):
                flush_until(f"s{j}hp{hp}")
                attention_block(j, hp,
                                fast_norm=(j == NSTRIP - 1 and i_hp >= HP2 - 2))
            if j + 2 < NSTRIP:
                load_strip(j + 2)
                filler.extend(strip_items(j + 2))
            if j < NSTRIP - 1:
                filler.extend(po_items(j))
        filler.extend(po_items(NSTRIP - 1, c4_order=(1, 2, 3, 0), act_copies=True))
        flush()

    if split_waits:
        _split_excess_waits(nc)
    return nc


def _get_program():
    global _PROGRAM
    if _PROGRAM is None:
        _PROGRAM = _build_program()
    return _PROGRAM


def _make_in_maps(x, Wk, Wq, Wv, Wp):
    import ml_dtypes
    bf16 = ml_dtypes.bfloat16
    x = np.asarray(x, dtype=np.float32)
    Wk = np.asarray(Wk, dtype=np.float32)
    Wq = np.asarray(Wq, dtype=np.float32)
    Wv = np.asarray(Wv, dtype=np.float32)
    Wp = np.asarray(Wp, dtype=np.float32)
    in_maps = []
    for core in range(8):
        b, g = core // GROUPS, core % GROUPS
        rows = slice(GC * g, GC * (g + 1))
        in_maps.append({
            "xT": np.ascontiguousarray(x[b].T).astype(bf16),          # [C, T]
            "wqT": np.ascontiguousarray(Wq[rows, :].T).astype(bf16),  # [C, GC]
            "wkT": np.ascontiguousarray(Wk[rows, :].T).astype(bf16),
            "wvT": np.ascontiguousarray(Wv[rows, :].T).astype(bf16),
            "wpT": np.ascontiguousarray(Wp[:, rows].T).astype(bf16),  # [GC, C]
        })
    return in_maps


def run(x, Wk, Wq, Wv, Wp, bp, trace=False, **spmd_kwargs):
    nc = _get_program()
    in_maps = _make_in_maps(x, Wk, Wq, Wv, Wp)
    res = run_bass_kernel_spmd(nc, in_maps, list(range(8)), trace=trace, **spmd_kwargs)
    bp = np.asarray(bp, dtype=np.float32)
    out = np.empty((B, T, C), dtype=np.float32)
    for b in range(B):
        out[b] = (np.asarray(res.results[GROUPS * b]["outp"], dtype=np.float32)
                  + np.asarray(res.results[GROUPS * b + 1]["outp"], dtype=np.float32) + bp)
    return out, res


def kernel(x, Wk, Wq, Wv, Wp, bp):
    out, _ = run(x, Wk, Wq, Wv, Wp, bp)
    return out


# revision 61
# speedup vs baseline: 1.0069x; 1.0069x over previous
"""Multi-head causal self-attention (B=4, T=2048, C=1024, 16 heads) on 8 trn2 cores.

Sharding: data-parallel over batch (4) x tensor-parallel over heads (2 groups of 8).
Core m handles batch m//2, head group m%2. Host pre-transposes x and the weights
(bf16) so every on-device matmul consumes operands in natural layout; the output
projection partial sums are pair-reduced on host (+bias).

v2 pipeline (bf16 matmuls, fp32 PSUM):
  - QKV projection strips interleaved with attention query-blocks at single-MM
    granularity: projection matmuls fill the PE bubbles that the ACT-bound
    softmax stretches would otherwise leave.
  - scores for a head pair land in the two banks of one [128,1024] PSUM tile;
    ONE activation (2D AP) exponentiates both banks -> halves ACT inst count.
  - causal masking: exp the unmasked column range, then gpsimd affine_select
    zeroes the diagonal triangle in the bf16 p tile (Pool engine is idle).
  - softmax denominators via a ones-column in v; normalization r=1/denom via
    DVE reciprocal, broadcast across partitions with a free-dim-stride-0 DMA,
    then one fused PSUM*SBUF->bf16 multiply per head writes avT in place
    (par1 writes partitions 64:128 directly - no staging DMA). The very last
    block swaps the DMA broadcast for a ones-matmul into the freed scores
    ring (shorter latency on the kernel's critical tail).
  - output projection chains rotate their accumulation order (c4 1,2,3,0 on
    the last query block) so only one matmul per chain gates on the final
    normalize; outputs stage through SBUF (DMA cannot read PSUM) with copies
    alternating ACT/DVE at the tail.
"""

import numpy as np

import concourse.bass as bass
import concourse.mybir as mybir
import concourse.tile as tile
from concourse.bass_utils import run_bass_kernel_spmd

F32 = mybir.dt.float32
F32R = mybir.dt.float32r
BF16 = mybir.dt.bfloat16
AF = mybir.ActivationFunctionType
MULT = mybir.AluOpType.mult

B, T, C = 4, 2048, 1024
HEADS, D = 16, 64
GROUPS = 2                  # head groups (tensor parallel)
HPC = HEADS // GROUPS       # heads per core = 8
GC = HPC * D                # group channel width = 512
CCH = C // 128              # contraction chunks = 8
NSTRIP = T // 512           # strips / query blocks = 4
HP2 = HPC // 2              # head pairs = 4

_PROGRAM = None


def _patch_drain_chunking():
    """The axon walrus build rejects instructions with >~4 sem waits; Tile's
    kernel-tail drain waits on every live semaphore at once. Split it into a
    chain of drains (excess waits then move onto NoOps via
    _split_excess_waits, keeping every instruction at <=1 wait)."""
    from bass_rust import VectorClock, ScopedClock

    if getattr(tile.TileContext, "_drain_chunk_patched", False):
        return

    def _drain_and_barrier(self, tick_clock, wait_clock):
        gc_vec = list(tick_clock.global_clock)
        nz = [i for i, t in enumerate(gc_vec) if t > 0]
        CHUNK = 32
        for k in range(0, len(nz), CHUNK):
            keep = set(nz[k:k + CHUNK])
            partial = [gc_vec[i] if i in keep else 0 for i in range(len(gc_vec))]
            d = self.nc.sync.drain()
            wait_clock.add_sem_waits(d.ins, ScopedClock({None: VectorClock(partial)}))
        self.nc.all_engine_barrier()
        assert self.sems is not None
        popped = self.nc._tile_sem_poison_stack.pop()
        assert popped is self._sem_poison
        self.nc.clear_and_free_semaphores(list(self.sems.allocated().values()))
        self.nc.all_engine_barrier()

    tile.TileContext._drain_and_barrier = _drain_and_barrier
    tile.TileContext._drain_chunk_patched = True


def _split_excess_waits(nc, maxw=1, maxw_other=None):
    """Walrus rejects instructions carrying more than ~1 sem wait. Move excess
    waits onto same-engine NoOps inserted immediately before the instruction
    (engine streams execute in bb order, so semantics are preserved)."""
    from bass_rust import InstNoOp

    ctr = 0
    for f in nc.m.functions:
        for bb in f.blocks:
            new_insts = []
            for inst in bb.instructions:
                si = inst.sync_info
                waits = list(si.on_wait) if si and si.on_wait else []
                lim = maxw
                if maxw_other is not None and str(inst.engine) != 'EngineType.PE':
                    lim = maxw_other
                maxw_eff = lim
                if len(waits) > maxw_eff:
                    head, rest = waits[:-maxw_eff], waits[-maxw_eff:]
                    for k in range(0, len(head), maxw_eff):
                        ctr += 1
                        new_insts.append(InstNoOp(
                            name=f"waitnop_{ctr}",
                            engine=inst.engine,
                            sync_info=mybir.SyncInfo(
                                on_wait=head[k:k + maxw_eff], on_update=[]),
                        ))
                    inst.sync_info = mybir.SyncInfo(on_wait=rest, on_update=si.on_update)
                new_insts.append(inst)
            bb.instructions = new_insts
    return ctr


def _build_program(split_waits=True):
    _patch_drain_chunking()
    nc = bass.Bass()

    xT_d = nc.declare_dram_parameter("xT", [C, T], BF16, isOutput=False)
    wq_d = nc.declare_dram_parameter("wqT", [C, GC], BF16, isOutput=False)
    wk_d = nc.declare_dram_parameter("wkT", [C, GC], BF16, isOutput=False)
    wv_d = nc.declare_dram_parameter("wvT", [C, GC], BF16, isOutput=False)
    wp_d = nc.declare_dram_parameter("wpT", [GC, C], BF16, isOutput=False)
    out_d = nc.declare_dram_parameter("outp", [T, C], F32, isOutput=True)

    from collections import deque
    from contextlib import ExitStack

    with tile.TileContext(nc) as tc, ExitStack() as stack:
        pers = stack.enter_context(tc.tile_pool(name="pers", bufs=1))
        qT = pers.tile([128, HP2, T], BF16, tag="qT")     # [chan-in-pair, hp, t]
        kT = pers.tile([128, HP2, T], BF16, tag="kT")
        # v with a trailing ones column per head: [key-chunk, head, D+1]
        v = pers.tile([128, T // 128, HPC, D + 1], BF16, tag="v")
        avT = pers.tile([128, HP2, T], BF16, tag="avT")
        wq = pers.tile([128, CCH, GC], BF16, tag="wq")
        wk = pers.tile([128, CCH, GC], BF16, tag="wk")
        wv = pers.tile([128, CCH, GC], BF16, tag="wv")
        wp = pers.tile([128, GC // 128, C], BF16, tag="wp")

        nc.gpsimd.memset(v[:, :, :, D:D + 1], 1.0)
        # walrus: every writer of a tensor consumed by an f32r matmul must
        # itself round to f32r (ACT/DMA only) -> keep Pool/DVE-written scratch
        # in separate tiles from the f32r-consumed ones
        ones_src = pers.tile([128, 64], F32, tag="ones_src")
        nc.gpsimd.memset(ones_src[64:65, :], 1.0)
        ones = pers.tile([128, 64], F32, tag="ones")
        nc.scalar.activation(ones[64:65, :].bitcast(F32R), ones_src[64:65, :],
                             AF.Copy, scale=1.0)

        warm_w = pers.tile([128, 64], BF16, tag="warm_w")
        nc.gpsimd.memset(warm_w[:, :], 0.0)

        xs_pool = stack.enter_context(tc.tile_pool(name="xs", bufs=4))
        pt_pool = stack.enter_context(tc.tile_pool(name="pt", bufs=6))
        rb_pool = stack.enter_context(tc.tile_pool(name="rb", bufs=3))
        ob_pool = stack.enter_context(tc.tile_pool(name="ob", bufs=4))
        ps = stack.enter_context(tc.tile_pool(name="ps", bufs=1, space="PSUM"))

        # ---- initial DMAs ----
        xs_tiles = [None] * NSTRIP

        def load_strip(s):
            xs = xs_pool.tile([128, CCH, 512], BF16, tag="xs", name=f"xs{s}")
            nc.sync.dma_start(
                xs[:, :, :],
                xT_d[:, 512 * s:512 * (s + 1)].rearrange("(c p) t -> p c t", p=128))
            xs_tiles[s] = xs

        # first-use order, with wk/xs0 halved so the k chain starts ASAP
        xs0 = xs_pool.tile([128, CCH, 512], BF16, tag="xs", name="xs0")
        xT0 = xT_d[:, 0:512].rearrange("(c p) t -> p c t", p=128)
        wkT0 = wk_d[:, :].rearrange("(c p) o -> p c o", p=128)
        wqT0 = wq_d[:, :].rearrange("(c p) o -> p c o", p=128)
        nc.sync.dma_start(wk[:, 0:4, :], wkT0[:, 0:4, :])
        nc.sync.dma_start(xs0[:, 0:4, :], xT0[:, 0:4, :])
        nc.sync.dma_start(wk[:, 4:8, :], wkT0[:, 4:8, :])
        nc.sync.dma_start(xs0[:, 4:8, :], xT0[:, 4:8, :])
        nc.sync.dma_start(wq[:, 0:4, :], wqT0[:, 0:4, :])
        nc.sync.dma_start(wq[:, 4:8, :], wqT0[:, 4:8, :])
        xs_tiles[0] = xs0
        # keep the PE pstate warm through the initial load: tiny matmuls that
        # each consume a just-landed DMA chunk (same reason real kernels avoid
        # >3us PE-idle gaps: the clock ramps back down)
        warmav = ps.tile([128, 1024], F32, tag="av", bufs=1, name="warmav")
        for wsrc in (wk[:, 0, 0:64], xs0[:, 0, 0:64], wk[:, 4, 0:64],
                     xs0[:, 4, 0:64], wq[:, 0, 0:64], wq[:, 4, 0:64]):
            nc.tensor.matmul(warmav[0:64, 0:64], warm_w[:, :], wsrc,
                             start=True, stop=True)
        wvT0 = wv_d[:, :].rearrange("(c p) o -> p c o", p=128)
        nc.sync.dma_start(wv[:, 0:4, :], wvT0[:, 0:4, :])
        nc.sync.dma_start(wv[:, 4:8, :], wvT0[:, 4:8, :])
        nc.sync.dma_start(wp[:, :, :],
                          wp_d[:, :].rearrange("(c p) o -> p c o", p=128))

        # ---- phase-1 / phase-3 micro-item generators (PE filler) ----
        def qk_chain_items(s, w_sb, dst, o):
            cell = {}
            def mk_mm(c):
                def it():
                    if c == 0:
                        cell["p"] = ps.tile([128, 512], F32, tag="p1", bufs=2, name="pq")
                    nc.tensor.matmul(cell["p"][:, :], w_sb[:, c, 128 * o:128 * (o + 1)],
                                     xs_tiles[s][:, c, :],
                                     start=(c == 0), stop=(c == CCH - 1))
                return it
            for c in range(CCH):
                yield mk_mm(c)
            def cp():
                nc.vector.tensor_copy(dst[:, o, 512 * s:512 * (s + 1)], cell["p"][:, :])
            yield cp

        def v_chain_items(s, tt):
            cell = {}
            def mk_mm(c):
                def it():
                    if c == 0:
                        cell["p"] = ps.tile([128, 512], F32, tag="p1", bufs=2, name="pv")
                    nc.tensor.matmul(cell["p"][:, :], xs_tiles[s][:, c, 128 * tt:128 * (tt + 1)],
                                     wv[:, c, :], start=(c == 0), stop=(c == CCH - 1))
                return it
            for c in range(CCH):
                yield mk_mm(c)
            def cp():
                # ACT copy (same act table as Exp): keeps DVE free during the
                # strip dumps where these chains run back-to-back
                nc.scalar.activation(
                    v[:, 4 * s + tt, :, 0:D],
                    cell["p"][:, :].rearrange("p (h d) -> p h d", h=HPC),
                    AF.Copy, scale=1.0)
            yield cp

        def strip_items(s):
            # hp0's k/q chains + all v chains first (attention j=s, hp=0 needs
            # them); later head pairs' chains trail as PE filler with per-hp
            # barrier markers. Strip 0 front-loads the k chains: they only
            # need wk+xs0, so they fill the PE while wq/wv are still in DMA.
            if s == 0:
                yield from qk_chain_items(s, wk, kT, 0)
                yield from qk_chain_items(s, wk, kT, 1)
                yield from qk_chain_items(s, wq, qT, 0)
                yield from qk_chain_items(s, wq, qT, 1)
                for tt in range(4):
                    yield from v_chain_items(s, tt)
                yield "s0hp0"
                yield "s0hp1"
                for o in range(2, HP2):
                    yield from qk_chain_items(s, wk, kT, o)
                    yield from qk_chain_items(s, wq, qT, o)
                    yield f"s0hp{o}"
                return
            yield from qk_chain_items(s, wk, kT, 0)
            yield from qk_chain_items(s, wq, qT, 0)
            for tt in range(4):
                yield from v_chain_items(s, tt)
            yield f"s{s}hp0"
            for o in range(1, HP2):
                yield from qk_chain_items(s, wk, kT, o)
                yield from qk_chain_items(s, wq, qT, o)
                yield f"s{s}hp{o}"

        def po_items(j, c4_order=(0, 1, 2, 3), act_copies=False, tts=None):
            for tt in (tts if tts is not None else range(4 * j, 4 * (j + 1))):
                for o2 in range(2):
                    cell = {}
                    def mk_mm(ci, c4, tt=tt, o2=o2, cell=cell):
                        def it():
                            if ci == 0:
                                cell["p"] = ps.tile([128, 512], F32, tag="p1",
                                                    bufs=2, name="po")
                            nc.tensor.matmul(cell["p"][:, :],
                                             avT[:, c4, 128 * tt:128 * (tt + 1)],
                                             wp[:, c4, 512 * o2:512 * (o2 + 1)],
                                             start=(ci == 0), stop=(ci == GC // 128 - 1))
                        return it
                    for ci, c4 in enumerate(c4_order):
                        yield mk_mm(ci, c4)
                    def cp(cell=cell, tt=tt, o2=o2):
                        cell["ob"] = ob_pool.tile([128, 512], F32, tag="ob", name="ob")
                        if act_copies and (tt + o2) % 2 == 0:
                            nc.scalar.activation(cell["ob"][:, :], cell["p"][:, :],
                                                 AF.Copy, scale=1.0)
                        else:
                            nc.vector.tensor_copy(cell["ob"][:, :], cell["p"][:, :])
                    yield cp
                    def dma(tt=tt, o2=o2, cell=cell):
                        nc.sync.dma_start(
                            out_d[128 * tt:128 * (tt + 1), 512 * o2:512 * (o2 + 1)],
                            cell["ob"][:, :])
                    yield dma

        # PE filler queue. Items run strictly in queue order (chains allocate
        # PSUM ring slots, so partial-chain reordering would deadlock);
        # markers let the schedule force "everything up to X done" barriers.
        filler = deque()
        seen_marks = set()

        def mark(name):
            filler.append(name)

        def pull(n):
            done = 0
            while filler and done < n:
                it = filler.popleft()
                if isinstance(it, str):
                    seen_marks.add(it)
                    continue
                it()
                done += 1

        def flush_until(name):
            while name not in seen_marks:
                assert filler, f"marker {name} never queued"
                it = filler.popleft()
                if isinstance(it, str):
                    seen_marks.add(it)
                else:
                    it()

        def flush():
            while filler:
                it = filler.popleft()
                if isinstance(it, str):
                    seen_marks.add(it)
                else:
                    it()

        # ---- attention for one (j, hp) block ----
        def attention_block(j, hp, fast_norm=False):
            nkc = 4 * (j + 1)
            av = ps.tile([128, 1024], F32, tag="av", bufs=1, name="av")

            def emit_scores_exp(i):
                roff = max(0, 128 * i - 512 * j)
                sp = ps.tile([128, 1024], F32, tag="s", bufs=2, name="sp")
                for par in range(2):
                    nc.tensor.matmul(
                        sp[:, 512 * par + roff:512 * (par + 1)],
                        kT[64 * par:64 * (par + 1), hp, 128 * i:128 * (i + 1)],
                        qT[64 * par:64 * (par + 1), hp, 512 * j + roff:512 * (j + 1)],
                        start=True, stop=True)
                ptile = pt_pool.tile([128, 1024], BF16, tag="pt", name="ptile")
                if roff == 0:
                    nc.scalar.activation(ptile[:, :], sp[:, :], AF.Exp, scale=0.125)
                else:
                    nc.scalar.activation(
                        ptile[:, :].rearrange("p (g c) -> p g c", g=2)[:, :, roff:512],
                        sp[:, :].rearrange("p (g c) -> p g c", g=2)[:, :, roff:512],
                        AF.Exp, scale=0.125)
                if 128 * i >= 512 * j:  # diagonal chunk: zero the triangle above diag
                    sel = ptile[:, :].rearrange("p (g c) -> p g c", g=2)[:, :, roff:roff + 128]
                    nc.gpsimd.affine_select(
                        out=sel, in_=sel,
                        compare_op=mybir.AluOpType.is_ge, fill=0.0, base=0,
                        pattern=[[0, 2], [1, 128]], channel_multiplier=-1)
                return ptile, roff

            def emit_av(i, ptile, roff):
                for par in range(2):
                    nc.tensor.matmul(
                        av[0:D + 1, 512 * par + roff:512 * (par + 1)],
                        v[:, i, 2 * hp + par, :],
                        ptile[:, 512 * par + roff:512 * (par + 1)],
                        start=(i == 0), stop=(i == nkc - 1))

            prev = emit_scores_exp(0)
            for i in range(1, nkc):
                cur = emit_scores_exp(i)
                pull(1)
                emit_av(i - 1, *prev)
                prev = cur
            pull(1)
            emit_av(nkc - 1, *prev)

            # normalize: r = 1/denom (row 64), partition-broadcast, one fused
            # multiply per par writes avT directly
            rbt = rb_pool.tile([128, 1024], F32, tag="rb", name="rbt")
            if fast_norm:
                # tail path: the broadcast DMA's ~3us latency would sit bare
                # on the critical path; use a ones-matmul into the (now idle)
                # scores ring + ACT copy instead, pipelined across pars
                bc = ps.tile([128, 1024], F32, tag="s", bufs=2, name="bc")
                rr = rb_pool.tile([128, 1024], F32, tag="rr", bufs=1, name="rr")
                rf = rb_pool.tile([128, 1024], F32, tag="rf", bufs=1, name="rf")
                cols = [slice(512 * par, 512 * (par + 1)) for par in range(2)]
                for par in range(2):
                    nc.vector.reciprocal(rr[64:65, cols[par]], av[64:65, cols[par]])
                for par in range(2):
                    nc.scalar.activation(rf[64:65, cols[par]].bitcast(F32R),
                                         rr[64:65, cols[par]], AF.Copy, scale=1.0)
                for par in range(2):
                    nc.tensor.matmul(bc[0:64, cols[par]], ones[64:65, :].bitcast(F32R),
                                     rf[64:65, cols[par]].bitcast(F32R),
                                     start=True, stop=True)
                for par in range(2):
                    nc.scalar.activation(rbt[0:64, cols[par]], bc[0:64, cols[par]],
                                         AF.Copy, scale=1.0)
                for par in range(2):
                    nc.vector.tensor_tensor(
                        avT[64 * par:64 * (par + 1), hp, 512 * j:512 * (j + 1)],
                        av[0:D, cols[par]], rbt[0:64, cols[par]], op=MULT)
            else:
                nc.vector.reciprocal(rbt[64:65, :], av[64:65, :])
                nc.sync.dma_start(
                    rbt[0:64, :],
                    rbt[64:65, :].unsqueeze(1).to_broadcast((1, 64, 1024)))
                for par in range(2):
                    nc.vector.tensor_tensor(
                        avT[64 * par:64 * (par + 1), hp, 512 * j:512 * (j + 1)],
                        av[0:D, 512 * par:512 * (par + 1)],
                        rbt[0:64, 512 * par:512 * (par + 1)], op=MULT)

        # ---- schedule ----
        # A(j, hp) flushes the queue up to strip j's hp marker; everything
        # behind that marker (later head pairs' projection chains, the next
        # strip, deferred output-projection chains) serves as PE filler
        # inside the ACT-bound attention stretches.
        filler.extend(strip_items(0))
        load_strip(1)
        filler.extend(strip_items(1))

        # po(0) feeds the A(2)/A(3) pulls; po(1)/po(2) are held back so the
        # PE still has independent work while the last block's normalize
        # chain (reciprocal -> broadcast DMA -> avT multiplies) drains
        for j in range(NSTRIP):
            # at j=3, hp0 goes last: po(3) chains accumulate c4 in order
            # 1,2,3,0 so only their final matmul waits on the last
            # normalize chain of the kernel
            hps = (1, 2, 3, 0) if j == NSTRIP - 1 else range(HP2)
            for i_hp, hp in enumerate(hps):
                flush_until(f"s{j}hp{hp}")
                attention_block(j, hp,
                                fast_norm=(j == NSTRIP - 1 and i_hp == HP2 - 1))
            if j + 2 < NSTRIP:
                load_strip(j + 2)
                filler.extend(strip_items(j + 2))
            if j < NSTRIP - 1:
                filler.extend(po_items(j))
        filler.extend(po_items(NSTRIP - 1, c4_order=(1, 2, 3, 0), act_copies=True))
        flush()

    if split_waits:
        _split_excess_waits(nc)
    return nc


def _get_program():
    global _PROGRAM
    if _PROGRAM is None:
        _PROGRAM = _build_program()
    return _PROGRAM


def _make_in_maps(x, Wk, Wq, Wv, Wp):
    import ml_dtypes
    bf16 = ml_dtypes.bfloat16
    x = np.asarray(x, dtype=np.float32)
    Wk = np.asarray(Wk, dtype=np.float32)
    Wq = np.asarray(Wq, dtype=np.float32)
    Wv = np.asarray(Wv, dtype=np.float32)
    Wp = np.asarray(Wp, dtype=np.float32)
    in_maps = []
    for core in range(8):
        b, g = core // GROUPS, core % GROUPS
        rows = slice(GC * g, GC * (g + 1))
        in_maps.append({
            "xT": np.ascontiguousarray(x[b].T).astype(bf16),          # [C, T]
            "wqT": np.ascontiguousarray(Wq[rows, :].T).astype(bf16),  # [C, GC]
            "wkT": np.ascontiguousarray(Wk[rows, :].T).astype(bf16),
            "wvT": np.ascontiguousarray(Wv[rows, :].T).astype(bf16),
            "wpT": np.ascontiguousarray(Wp[:, rows].T).astype(bf16),  # [GC, C]
        })
    return in_maps


def run(x, Wk, Wq, Wv, Wp, bp, trace=False, **spmd_kwargs):
    nc = _get_program()
    in_maps = _make_in_maps(x, Wk, Wq, Wv, Wp)
    res = run_bass_kernel_spmd(nc, in_maps, list(range(8)), trace=trace, **spmd_kwargs)
    bp = np.asarray(bp, dtype=np.float32)
    out = np.empty((B, T, C), dtype=np.float32)
    for b in range(B):
        out[b] = (np.asarray(res.results[GROUPS * b]["outp"], dtype=np.float32)
                  + np.asarray(res.results[GROUPS * b + 1]["outp"], dtype=np.float32) + bp)
    return out, res


def kernel(x, Wk, Wq, Wv, Wp, bp):
    out, _ = run(x, Wk, Wq, Wv, Wp, bp)
    return out
